# revision 26
# baseline (speedup 1.0000x reference)
"""CRF loss kernel for Trainium2 (8 NeuronCores, data-parallel over batch).

Denominator via a forward/backward time split (512 serial steps instead
of 1024 per core):
  fwd:  a_t = exp(s_t + lnc) * (E^T a_{t-1}),  a_0 = exp(s_0+start+lnc)
        all 512 states dumped to HBM (bf16, 64-step blocks).
  bwd:  scores reversed and end-aligned per sequence on host;
        q_k = exp(s'_k + lnc) * (E q_{k-1}), q_0 = exp(s'_0+end+lnc).
  host: L<=512 -> lnZ = ln(dump[L-1]*exp(end)) + L*ln82
        L> 512 -> lnZ = ln(dump[L-513]*(E @ q_511)) + L*ln82
All recurrence matmuls bf16 [50x50] weights, 64-wide moving operand.
Numerator: per (b, chunk) ONE PE matmul against a host-packed
  [onehot|ohprev|pad|scores|ohL|d0] matrix accumulating C/D counts in
  PSUM, then one tensor_tensor_reduce against [trans; I; end; start];
  interleaved into the recurrence's PE stall gaps.
"""

import os
import numpy as np
import ml_dtypes

import concourse.bass as bass
import concourse.bacc as bacc
import concourse.mybir as mybir
from concourse import tile
from concourse.bass_utils import run_bass_kernel_spmd

B, S, T = 512, 1024, 50
NCORES = 8
BL = B // NCORES  # 64 sequences per core
HALF = S // 2     # 512 steps per direction
CONST = 82.0
LNC = np.float32(np.log(1.0 / CONST))

WCH = 32                    # steps per score chunk
NSCH = HALF // WCH          # 16 chunks per direction
DB = 64                     # steps per dump block
NDB = HALF // DB            # 8 dump blocks
NCH = 8                     # numerator chunks (128 rows each)
CW = 166                    # combo cols: oh|ohprev|pad|scores|ohL|d0

TRACE = os.environ.get("CRF_TRACE") == "1"

_cached = {}


def _build_nc():
    f32 = mybir.dt.float32
    bf16 = mybir.dt.bfloat16
    AF = mybir.ActivationFunctionType
    OP = mybir.AluOpType

    nc = bacc.Bacc(None, target_bir_lowering=False)

    # ---- DRAM I/O ----
    d_fsct = nc.dram_tensor("fsct", [T, HALF, BL], f32, kind="ExternalInput")
    d_bsct = nc.dram_tensor("bsct", [T, HALF, BL], f32, kind="ExternalInput")
    d_ewlog = nc.dram_tensor("ewlog", [T, 2 * T], f32, kind="ExternalInput")
    d_combo = nc.dram_tensor("combo", [2, NCH, 128, BL // 2, CW], bf16,
                             kind="ExternalInput")
    d_cin1 = nc.dram_tensor("cin1t", [T, 116], f32, kind="ExternalInput")
    d_ones = nc.dram_tensor("ones50", [T, 1], f32, kind="ExternalInput")

    d_fst = nc.dram_tensor("o_fst", [T, HALF * BL], bf16, kind="ExternalOutput")
    d_q = nc.dram_tensor("o_q", [T, BL], f32, kind="ExternalOutput")
    d_num = nc.dram_tensor("o_num", [BL, 1], f32, kind="ExternalOutput")

    with tile.TileContext(nc) as tc:
        with (
            tc.tile_pool(name="const", bufs=1) as cpool,
            tc.tile_pool(name="ring", bufs=4) as ring,
            tc.tile_pool(name="state", bufs=3) as spool,
            tc.tile_pool(name="work", bufs=2) as wpool,
            tc.tile_pool(name="ps_f", bufs=2, space="PSUM") as ps_f,
            tc.tile_pool(name="ps_b", bufs=2, space="PSUM") as ps_b,
            tc.tile_pool(name="ps_cd", bufs=2, space="PSUM") as ps_cd,
            tc.tile_pool(name="ps_misc", bufs=1, space="PSUM") as ps_misc,
        ):
            # ---- constants ----
            ewlog = cpool.tile([T, 2 * T], f32)
            nc.sync.dma_start(ewlog[:], d_ewlog[:])
            ew = cpool.tile([T, 2 * T], bf16)
            nc.scalar.activation(ew[:], ewlog[:], AF.Exp)

            cin1_dma = cpool.tile([T, 116], f32)
            nc.sync.dma_start(cin1_dma[:], d_cin1[:])
            cin1 = cpool.tile([T, 116], f32)
            nc.vector.tensor_copy(cin1[:], cin1_dma[:])
            ones50 = cpool.tile([T, 1], f32)
            nc.sync.dma_start(ones50[:], d_ones[:])

            # combo tiles: one batch-half resident at a time
            combos = {}

            def load_combo_ch(h, ch):
                ct = ring.tile([128, BL // 2, CW], bf16, tag=f"combo{ch}",
                               bufs=1, name=f"combo{ch}")
                nc.sync.dma_start(ct[:], d_combo[h, ch][:])
                combos[ch] = ct

            # ---- score chunk rings (exp'd in place) ----
            fchunks = {}
            bchunks = {}

            def ensure_chunk(which, m):
                store, dram, tag = ((fchunks, d_fsct, "fring")
                                    if which == "f" else
                                    (bchunks, d_bsct, "bring"))
                if m in store or m >= NSCH:
                    return
                tl = ring.tile([T, WCH, BL], f32, tag=tag)
                nc.sync.dma_start(tl[:], dram[:, m * WCH:(m + 1) * WCH, :])
                nc.scalar.activation(tl[:], tl[:], AF.Exp)
                store[m] = tl

            # recurrence chunks first: the serial chains must start ASAP,
            # the big combo DMAs stream in behind them.
            for m in range(3):
                ensure_chunk("f", m)
                ensure_chunk("b", m)
            for ch in range(NCH):
                load_combo_ch(0, ch)

            # ---- dump blocks (fwd states land here, then DMA out) ----
            dbt = [cpool.tile([T, DB * BL], bf16, name=f"dbt{i}")
                   for i in range(2)]

            def dump_slot(t):
                return dbt[(t // DB) % 2][:, (t % DB) * BL:(t % DB + 1) * BL]

            # ---- init states (ring chunks are already exp'd in place) ----
            nc.vector.tensor_copy(dump_slot(0), fchunks[0][:, 0, :])
            q0 = spool.tile([T, BL], bf16, tag="q")
            nc.vector.tensor_copy(q0[:], bchunks[0][:, 0, :])
            qcur = [q0]

            # ---- numerator work queue (interleaved into the loop) ----
            acc50 = cpool.tile([T, BL], f32)
            num_ops = []

            def make_num_ops():
                for h in range(2):
                    if h == 1:
                        # all half-1 combo loads must precede the first
                        # half-1 matmul: the mm closures read combos[ch]
                        # at pump time, so a later loadh would leave them
                        # on the stale half-0 tile
                        for ch in range(NCH):
                            num_ops.append(("loadh", ch))
                    for bb in range(BL // 2):
                        b = h * (BL // 2) + bb

                        def mk_mm(bb, ch):
                            def run(cd):
                                ct = combos[ch]
                                nc.tensor.matmul(
                                    cd[:], ct[:, bb, 0:T], ct[:, bb, T:CW],
                                    start=(ch == 0), stop=(ch == NCH - 1),
                                    skip_group_check=True,
                                )
                            return run

                        def mk_ttr(b):
                            def run(cd):
                                scr = wpool.tile([T, 116], f32, tag="ttr_scr",
                                                 name="ttr_scr")
                                nc.vector.scalar_tensor_tensor(
                                    scr[:], cd[:], 1.0, cin1[:],
                                    OP.mult, OP.mult,
                                    accum_out=acc50[:, b:b + 1],
                                )
                            return run

                        ops = [("new", b)] \
                            + [("mm", mk_mm(bb, ch)) for ch in range(NCH)] \
                            + [("ttr", mk_ttr(b))]
                        num_ops.extend(ops)

            make_num_ops()
            num_i = 0
            cur_cd = [None]

            # ops for b=32..39 run while half-1 combos still stream in;
            # throttle the pump there so stalled matmuls never fill the
            # PE wait queue (depth 4) and block the recurrence chain.
            SLOW_LO = 32 * 10                # first op index of b=32
            SLOW_HI = SLOW_LO + 8 * 11       # last op of b=39 (loadh+10)

            def pump_num(n_mm, t=0):
                nonlocal num_i
                mm_done = 0
                while num_i < len(num_ops) and mm_done < n_mm:
                    kind, payload = num_ops[num_i]
                    if (kind == "mm" and t % 8 != 0
                            and SLOW_LO <= num_i < SLOW_HI):
                        return
                    if kind == "new":
                        cur_cd[0] = ps_cd.tile([T, 116], f32, tag="cdps",
                                               name="cdps")
                    elif kind == "loadh":
                        load_combo_ch(1, payload)
                    else:
                        payload(cur_cd[0])
                        if kind == "mm":
                            mm_done += 1
                    num_i += 1

            # ---- the two recurrence chains ----
            for t in range(1, HALF):
                m = t // WCH
                if t % WCH == 0:
                    # two chunks of prefetch (~33us) rides out the combo
                    # DMA bursts without starving the chains
                    ensure_chunk("f", m + 2)
                    ensure_chunk("b", m + 2)

                pf = ps_f.tile([T, BL], f32, tag="pf", name="pf", bufs=1)
                nc.tensor.matmul(pf[:], ew[:, 0:T], dump_slot(t - 1),
                                 skip_group_check=True)
                pb = ps_b.tile([T, BL], f32, tag="pb", name="pb", bufs=1)
                nc.tensor.matmul(pb[:], ew[:, T:2 * T], qcur[0][:],
                                 skip_group_check=True)

                pump_num(1, t)

                nc.vector.scalar_tensor_tensor(
                    dump_slot(t), pf[:], 1.0, fchunks[m][:, t % WCH, :],
                    OP.mult, OP.mult)
                qn = spool.tile([T, BL], bf16, tag="q", name="q")
                nc.vector.scalar_tensor_tensor(
                    qn[:], pb[:], 1.0, bchunks[m][:, t % WCH, :],
                    OP.mult, OP.mult)
                qcur[0] = qn

                if t % DB == DB - 1:
                    j = t // DB
                    nc.sync.dma_start(
                        d_fst[:, j * DB * BL:(j + 1) * DB * BL],
                        dbt[j % 2][:])

                # retire chunks no longer needed
                if t % WCH == WCH - 1 and m - 1 in fchunks:
                    del fchunks[m - 1], bchunks[m - 1]

            pump_num(len(num_ops))

            # ---- final q out ----
            qf = cpool.tile([T, BL], f32)
            nc.scalar.copy(qf[:], qcur[0][:])
            nc.sync.dma_start(d_q[:], qf[:])

            # ---- numerator final: sum acc50 over partitions ----
            nm_ps = ps_misc.tile([BL, 1], f32, tag="misc", name="numps")
            nc.tensor.matmul(nm_ps[:], acc50[:], ones50[:],
                             skip_group_check=True)
            num_sb = cpool.tile([BL, 1], f32)
            nc.scalar.copy(num_sb[:], nm_ps[:])
            nc.sync.dma_start(d_num[:], num_sb[:])

    nc.compile()
    nc.finalize()
    return nc


def _host_inputs(token_scores, tags, token_mask, transitions,
                 start_transitions, end_transitions):
    ts = np.ascontiguousarray(token_scores, dtype=np.float32)
    tg = np.asarray(tags).astype(np.int64)
    mk = np.asarray(token_mask).astype(np.float32)
    tr = np.asarray(transitions, dtype=np.float32)
    st = np.asarray(start_transitions, dtype=np.float32)
    en = np.asarray(end_transitions, dtype=np.float32)
    L = np.asarray(token_mask).astype(np.int64).sum(1)

    # shared (replicated) constants
    ewlog = np.concatenate([tr, tr.T], axis=1).astype(np.float32)  # [T, 2T]
    cin1 = np.zeros((116, T), np.float32)
    cin1[0:T] = tr
    cin1[64:114] = np.eye(T, dtype=np.float32)
    cin1[114] = en
    cin1[115] = st
    cin1t = np.ascontiguousarray(cin1.T)          # [T, 116]
    ones50 = np.ones((T, 1), np.float32)

    ohl_full = mk - np.concatenate([mk[:, 1:], np.zeros((B, 1), np.float32)], 1)

    in_maps = []
    HB = BL // 2
    for r in range(NCORES):
        sl = slice(r * BL, (r + 1) * BL)
        tsc, tgc, mkc, ohlc, Lc = ts[sl], tg[sl], mk[sl], ohl_full[sl], L[sl]

        # fwd scores [T, HALF, BL]: col t = s_t + lnc (+start at t=0)
        fsct = tsc[:, 0:HALF, :].transpose(2, 1, 0) + LNC
        fsct[:, 0, :] += st[:, None]
        fsct = np.ascontiguousarray(fsct, np.float32)

        # bwd scores: col k = s_{L-1-k} + lnc (+end at k=0); zero pad
        kk = np.arange(HALF)
        idx = Lc[:, None] - 1 - kk[None, :]               # [BL, HALF]
        valid = idx >= 0
        idxc = np.clip(idx, 0, S - 1)
        gath = np.take_along_axis(tsc, idxc[:, :, None], axis=1)  # [BL,HALF,T]
        gath = np.where(valid[:, :, None], gath + LNC, LNC)
        bsct = gath.transpose(2, 1, 0)
        bsct[:, 0, :] += en[:, None]
        bsct = np.ascontiguousarray(bsct, np.float32)

        # numerator combo packing (merged single-matmul layout)
        oh = np.zeros((S, BL, T), np.float32)
        sidx = np.arange(S)
        bidx = np.arange(BL)
        oh[sidx[:, None], bidx[None, :], tgc[:, :].T] = 1.0
        oh *= mkc.T[:, :, None]
        ohprev = np.zeros_like(oh)
        ohprev[1:] = oh[:-1]
        combo = np.zeros((2, NCH, 128, HB, CW), np.float32)
        for h in range(2):
            bs = slice(h * HB, (h + 1) * HB)
            for ch in range(NCH):
                tt = slice(128 * ch, 128 * (ch + 1))
                combo[h, ch, :, :, 0:T] = oh[tt, bs, :]
                combo[h, ch, :, :, T:2 * T] = ohprev[tt, bs, :]
                combo[h, ch, :, :, 114:164] = \
                    tsc[bs, tt, :].transpose(1, 0, 2)
                combo[h, ch, :, :, 164] = ohlc[bs, tt].T
            combo[h, 0, 0, :, 165] = 1.0
        combo = combo.astype(ml_dtypes.bfloat16)

        in_maps.append({
            "fsct": fsct,
            "bsct": bsct,
            "ewlog": ewlog,
            "combo": combo,
            "cin1t": cin1t,
            "ones50": ones50,
        })
    return in_maps


def kernel(token_scores, tags, token_mask, transitions,
           start_transitions, end_transitions):
    if "nc" not in _cached:
        _cached["nc"] = _build_nc()
    nc = _cached["nc"]

    in_maps = _host_inputs(token_scores, tags, token_mask, transitions,
                           start_transitions, end_transitions)
    res = run_bass_kernel_spmd(nc, in_maps, list(range(NCORES)), trace=TRACE)
    if TRACE and res.exec_time_ns is not None:
        _cached["exec_time_ns"] = res.exec_time_ns
        print(f"HW exec time: {res.exec_time_ns} ns")

    _cached['res'] = res
    L = np.asarray(token_mask).astype(np.int64).sum(1)
    tr64 = np.asarray(transitions, np.float64)
    en64 = np.asarray(end_transitions, np.float64)
    E64 = np.exp(tr64)
    ene = np.exp(en64)
    lnC = np.log(np.float64(CONST))

    total = np.float64(0.0)
    for r in range(NCORES):
        out = res.results[r]
        num = out["o_num"].reshape(BL).astype(np.float64)
        dump = np.asarray(out["o_fst"]).astype(np.float64) \
            .reshape(T, HALF, BL)
        q = np.asarray(out["o_q"]).astype(np.float64).reshape(T, BL)
        Lc = L[r * BL:(r + 1) * BL]

        lnZ = np.zeros(BL)
        for b in range(BL):
            if Lc[b] <= HALF:
                dot = dump[:, Lc[b] - 1, b] @ ene
            else:
                dot = dump[:, Lc[b] - HALF - 1, b] @ (E64 @ q[:, b])
            lnZ[b] = np.log(dot) + Lc[b] * lnC
        total += np.sum(num - lnZ)
    loss = -(total / B)
    return np.array(loss, dtype=np.float32)
